# revision 34
# baseline (speedup 1.0000x reference)
"""AttentionFuserV3 Trainium2 kernel: 8-core pure data parallel over batch.

Reference computation per batch item x_b [L=1024, D=512]:
  stage1: q = x W1^T; S = q x^T; A = softmax(S); mix = A x;
          h = tanh([mix, q] Wo1^T); h = h / max(||h||_2, eps)     (per row)
  stage2: c = [h, x]; q2 = c W2^T; S2 = q2 c^T; A2 = softmax(S2);
          mix2 = A2 c; o = [mix2, q2] Wo2^T; emb = mean_l(o)

Pooling algebra: emb = mean_l(o) is linear, so the full [L,2D] mix2 and
[L,D] output projection are never materialized.  Instead
  emb = [colsum(A2) c, colsum(q2)] (Wo2^T / L)
where colsum(A2)[m] = sum_l exp(S2[l,m]) / denom[l] is a cheap
multiply+reduce over the already-computed exp tiles.  This removes the
two largest matmul groups of stage 2.

Layout strategy ("T-space"): all big tensors are kept transposed in SBUF
(feature dim on partitions, sequence dim L on the free axis) so every
matmul contraction lands on the partition axis.  Softmax runs without
max-subtraction (|scores| < ~70, exp stays in range); the denominator is
accumulated with a ones-vector matmul and applied as a column broadcast
produced by a rank-1 matmul.

All matmul operands are bf16 (same 1 cycle/row PE speed as f32r, half
the SBUF/DMA) which leaves room to double-buffer the per-batch tiles so
consecutive batches overlap.  Accumulation stays in f32 PSUM.
"""

import sys

sys.path.insert(0, "/opt/trn_rl_repo")

import numpy as np

N_GLOBAL, L, D = 32, 1024, 512
NCORES = 8
B = N_GLOBAL // NCORES          # 4 batch items per core
P = 128
LC = 512                        # l-chunk (matmul moving free dim)
NLC = L // LC                   # 2
DT = D // P                     # 4
LT = L // P                     # 8
D2T = 2 * D // P                # 8
C2T = 4 * D // P                # 16

_CACHE = {}


def _build_nc():
    import concourse.bass as bass  # noqa: F401
    import concourse.mybir as mybir
    import concourse.tile as tile
    from concourse import bacc

    f32 = mybir.dt.float32
    bf16 = mybir.dt.bfloat16
    AF = mybir.ActivationFunctionType
    ALU = mybir.AluOpType

    nc = bacc.Bacc("TRN2", target_bir_lowering=False, debug=False,
                   num_devices=NCORES)

    x_ext = nc.declare_dram_parameter("x", [B, L, D], bf16, isOutput=False)
    xT_ext = nc.declare_dram_parameter("xT", [B, D, L], bf16, isOutput=False)
    w1t_ext = nc.declare_dram_parameter("w1t", [D, D], bf16, isOutput=False)
    wo1t_ext = nc.declare_dram_parameter("wo1t", [2 * D, D], bf16, isOutput=False)
    w2t_ext = nc.declare_dram_parameter("w2t", [2 * D, 2 * D], bf16, isOutput=False)
    wo2t_ext = nc.declare_dram_parameter("wo2t", [4 * D, D], bf16, isOutput=False)
    id_ext = nc.declare_dram_parameter("ident", [P, P], bf16, isOutput=False)
    onc_ext = nc.declare_dram_parameter("onesc", [P, 1], bf16, isOutput=False)
    onr_ext = nc.declare_dram_parameter("onesr", [1, P], bf16, isOutput=False)
    # DRAM bounce buffer for the pooled-mix row -> column transpose
    mscr_ext = nc.declare_dram_parameter("mscr", [B, 2 * D], f32, isOutput=True)
    out_ext = nc.declare_dram_parameter("out", [B, D], f32, isOutput=True)

    import time as _time
    _t0 = _time.time()
    with tile.TileContext(nc) as tc:
        with tc.tile_pool(name="wp", bufs=1) as wp, \
             tc.tile_pool(name="cp", bufs=1) as cp, \
             tc.tile_pool(name="xp", bufs=2) as xp, \
             tc.tile_pool(name="hp", bufs=2) as hp, \
             tc.tile_pool(name="tp", bufs=2) as tp, \
             tc.tile_pool(name="vp", bufs=2) as vp, \
             tc.tile_pool(name="ep", bufs=1) as ep, \
             tc.tile_pool(name="ps", bufs=8, space="PSUM") as pp:

            # ---- weights (resident) ----
            w1t_s = wp.tile([P, DT, D], bf16, tag="w1t")
            nc.sync.dma_start(out=w1t_s, in_=w1t_ext.rearrange("(k p) e -> p k e", p=P))
            wo1t_s = wp.tile([P, D2T, D], bf16, tag="wo1t")
            nc.sync.dma_start(out=wo1t_s, in_=wo1t_ext.rearrange("(k p) e -> p k e", p=P))
            w2t_s = wp.tile([P, D2T, 2 * D], bf16, tag="w2t")
            nc.sync.dma_start(out=w2t_s, in_=w2t_ext.rearrange("(k p) e -> p k e", p=P))
            wo2t_s = wp.tile([P, C2T, D], bf16, tag="wo2t")
            nc.sync.dma_start(out=wo2t_s, in_=wo2t_ext.rearrange("(k p) e -> p k e", p=P))

            # ---- constants (DMA'd from host) ----
            ident_s = cp.tile([P, P], bf16, tag="ident")
            nc.sync.dma_start(out=ident_s, in_=id_ext[:, :])
            ones_s = cp.tile([P, 1], bf16, tag="ones")
            nc.sync.dma_start(out=ones_s, in_=onc_ext[:, :])
            onesr_s = cp.tile([1, P], bf16, tag="onesr")
            nc.sync.dma_start(out=onesr_s, in_=onr_ext[:, :])

            def mm(out, lhsT, rhs, first, last):
                nc.tensor.matmul(out, lhsT, rhs, start=first, stop=last)

            def bcast_recip(denom_ps, clamp_eps=None):
                """[1,512] PSUM denominator -> [128,512] SBUF broadcast of its
                reciprocal (optionally sqrt+clamp first)."""
                rv = vp.tile([1, LC], bf16, tag="rv", bufs=2)
                with nc.allow_low_precision(reason="bf16 softmax scale"):
                    if clamp_eps is not None:
                        nv = vp.tile([1, LC], f32, tag="nv", bufs=2)
                        nc.scalar.sqrt(nv, denom_ps[0:1, :])
                        nc.vector.tensor_scalar_max(nv, nv, clamp_eps)
                        nc.vector.reciprocal(rv, nv)
                    else:
                        nc.vector.reciprocal(rv, denom_ps[0:1, :])
                ps_b = pp.tile([P, LC], f32, tag="ps")
                mm(ps_b, onesr_s, rv[0:1, :], True, True)
                bc = vp.tile([P, LC], f32, tag="bc", bufs=4)
                nc.scalar.copy(bc, ps_b)
                return bc

            for b in range(B):
                xT_s = xp.tile([P, DT, L], bf16, tag="xT")
                nc.sync.dma_start(out=xT_s, in_=xT_ext[b].rearrange("(k p) l -> p k l", p=P))
                x_s = xp.tile([P, LT, D], bf16, tag="x")
                nc.sync.dma_start(out=x_s, in_=x_ext[b].rearrange("(k p) d -> p k d", p=P))
                hTn_s = hp.tile([P, DT, L], bf16, tag="hTn")

                # ================= stage 1 =================
                for lc in range(NLC):
                    ls = slice(lc * LC, (lc + 1) * LC)

                    # ph1: qT[e,l] = W1T^T-contraction over d
                    qT_s = tp.tile([P, DT, LC], bf16, tag="qt")
                    for et in range(DT):
                        ps = pp.tile([P, LC], f32, tag="ps")
                        for dk in range(DT):
                            mm(ps, w1t_s[:, dk, et * P:(et + 1) * P],
                               xT_s[:, dk, ls], dk == 0, dk == DT - 1)
                        nc.scalar.copy(qT_s[:, et, :], ps)

                    # ph2: scoresT[m,l] -> exp -> denom
                    expT_s = tp.tile([P, LT, LC], bf16, tag="exp")
                    ps_d = pp.tile([P, LC], f32, tag="ps")
                    for mt in range(LT):
                        ps = pp.tile([P, LC], f32, tag="ps")
                        for ek in range(DT):
                            mm(ps, xT_s[:, ek, mt * P:(mt + 1) * P],
                               qT_s[:, ek, :], ek == 0, ek == DT - 1)
                        nc.scalar.activation(expT_s[:, mt, :], ps, AF.Exp)
                        mm(ps_d[0:1, :], ones_s, expT_s[:, mt, :],
                           mt == 0, mt == LT - 1)
                    bc1 = bcast_recip(ps_d)

                    # ph3: mixT'[d,l] = sum_m x[m,d] expT[m,l], then normalize
                    mixT_s = tp.tile([P, DT, LC], bf16, tag="mix")
                    ps_m = [pp.tile([P, LC], f32, tag="ps", name=f"psm_{b}_{lc}_{i}")
                            for i in range(DT)]
                    for mk in range(LT):
                        for dt in range(DT):
                            mm(ps_m[dt], x_s[:, mk, dt * P:(dt + 1) * P],
                               expT_s[:, mk, :], mk == 0, mk == LT - 1)
                    for dt in range(DT):
                        nc.vector.tensor_mul(mixT_s[:, dt, :], ps_m[dt], bc1)

                    # ph4: out1T[o,l] = Wo1T-contraction over c=[mix,q]; tanh
                    hT_s = tp.tile([P, DT, LC], bf16, tag="ht")
                    for ot in range(DT):
                        ps = pp.tile([P, LC], f32, tag="ps")
                        for ck in range(D2T):
                            rhs = mixT_s[:, ck, :] if ck < DT else qT_s[:, ck - DT, :]
                            mm(ps, wo1t_s[:, ck, ot * P:(ot + 1) * P],
                               rhs, ck == 0, ck == D2T - 1)
                        nc.scalar.activation(hT_s[:, ot, :], ps, AF.Tanh)

                    # ph5: L2 norm over d (partition axis) via ones-matmul
                    hsq_s = tp.tile([P, DT, LC], bf16, tag="qt", name=f"hsq_{b}_{lc}")
                    for dt in range(DT):
                        nc.scalar.activation(hsq_s[:, dt, :], hT_s[:, dt, :], AF.Square)
                    ps_n = pp.tile([P, LC], f32, tag="ps")
                    for dt in range(DT):
                        mm(ps_n[0:1, :], ones_s, hsq_s[:, dt, :], dt == 0, dt == DT - 1)
                    bc2 = bcast_recip(ps_n, clamp_eps=1e-12)
                    for dt in range(DT):
                        nc.vector.tensor_mul(hTn_s[:, dt, ls], hT_s[:, dt, :], bc2)

                # ph6: transpose hidden_norm back to natural layout.
                # Reuses the now-dead stage-1 qt/ht pool slots for hn.
                hn_a = tp.tile([P, LT // 2, D], bf16, tag="qt", name=f"hna_{b}")
                hn_b = tp.tile([P, LT // 2, D], bf16, tag="ht", name=f"hnb_{b}")

                def hn_nat(lt):
                    return hn_a[:, lt, :] if lt < LT // 2 else hn_b[:, lt - LT // 2, :]

                for lt in range(LT):
                    for dt in range(DT):
                        ps_t = pp.tile([P, P], bf16, tag="ps", name=f"pst_{b}_{lt}_{dt}")
                        nc.tensor.transpose(
                            ps_t, hTn_s[:, dt, lt * P:(lt + 1) * P], ident_s)
                        nc.scalar.copy(hn_nat(lt)[:, dt * P:(dt + 1) * P], ps_t)

                # ================= stage 2 =================
                a2s_s = vp.tile([P, LT], f32, tag="a2s", bufs=2)
                a2p_s = vp.tile([P, LT, NLC], f32, tag="a2p", bufs=2)
                q2r_s = vp.tile([P, D2T, NLC], f32, tag="q2r", bufs=2)
                comb_s = vp.tile([P, C2T], f32, tag="comb", bufs=2)
                scr_s = vp.tile([P, LC], f32, tag="scr", bufs=2)
                for lc in range(NLC):
                    ls = slice(lc * LC, (lc + 1) * LC)

                    def c2T(k, fslice):
                        """combined2T[d2,·] k-tile: [hTn; xT]"""
                        return (hTn_s[:, k, fslice] if k < DT
                                else xT_s[:, k - DT, fslice])

                    # ph7: q2T[e2,l]; column-sum of q2 over l (free axis),
                    # reduced from PSUM
                    q2T_s = tp.tile([P, D2T, LC], bf16, tag="q2")
                    for et in range(D2T):
                        ps = pp.tile([P, LC], f32, tag="ps")
                        for dk in range(D2T):
                            mm(ps, w2t_s[:, dk, et * P:(et + 1) * P],
                               c2T(dk, ls), dk == 0, dk == D2T - 1)
                        nc.scalar.copy(q2T_s[:, et, :], ps)
                        nc.vector.tensor_reduce(q2r_s[:, et, lc:lc + 1], ps,
                                                axis=mybir.AxisListType.X,
                                                op=ALU.add)

                    # ph8: scores2T -> exp2 -> denom2
                    exp2T_s = tp.tile([P, LT, LC], bf16, tag="exp",
                                      name=f"exp2_{b}_{lc}")
                    ps_d = pp.tile([P, LC], f32, tag="ps")
                    for mt in range(LT):
                        ps = pp.tile([P, LC], f32, tag="ps")
                        for ek in range(D2T):
                            mm(ps, c2T(ek, slice(mt * P, (mt + 1) * P)),
                               q2T_s[:, ek, :], ek == 0, ek == D2T - 1)
                        nc.scalar.activation(exp2T_s[:, mt, :], ps, AF.Exp)
                        mm(ps_d[0:1, :], ones_s, exp2T_s[:, mt, :],
                           mt == 0, mt == LT - 1)
                    bc3 = bcast_recip(ps_d)

                    # ph9: A2 column sums a2s[m] = sum_l exp2T[m,l]/denom[l]
                    for mt in range(LT):
                        nc.vector.tensor_mul(scr_s, exp2T_s[:, mt, :], bc3)
                        nc.vector.tensor_reduce(a2p_s[:, mt, lc:lc + 1], scr_s,
                                                axis=mybir.AxisListType.X,
                                                op=ALU.add)

                # ph10: pooled epilogue.
                # comb = [colsum(A2) @ c, colsum(q2)] as a [4D] column vector.
                nc.vector.tensor_add(comb_s[:, D2T:C2T], q2r_s[:, :, 0],
                                     q2r_s[:, :, 1])
                nc.vector.tensor_add(a2s_s, a2p_s[:, :, 0], a2p_s[:, :, 1])
                a2sr_s = vp.tile([P, LT], bf16, tag="a2sr", bufs=2)
                nc.scalar.copy(a2sr_s, a2s_s)
                # colsum(A2) @ c: two [1,D] row matmuls over the natural-layout
                # halves of c = [hn, x]; rows become comb columns via a DRAM
                # bounce (cross-partition move)
                for half in range(2):
                    ps_mx = pp.tile([1, D], f32, tag="ps", name=f"psmx_{b}_{half}")
                    for mk in range(LT):
                        rhs = hn_nat(mk) if half == 0 else x_s[:, mk, :]
                        mm(ps_mx[0:1, :], a2sr_s[:, mk:mk + 1], rhs,
                           mk == 0, mk == LT - 1)
                    mrow = vp.tile([1, D], f32, tag="mrow", bufs=2,
                                   name=f"mrow_{b}_{half}")
                    nc.scalar.copy(mrow, ps_mx)
                    nc.sync.dma_start(
                        out=mscr_ext[b:b + 1, half * D:(half + 1) * D],
                        in_=mrow[0:1, :])
                nc.sync.dma_start(
                    out=comb_s[:, 0:D2T],
                    in_=mscr_ext[b].rearrange("(k p) -> p k", p=P))
                combr_s = vp.tile([P, C2T], bf16, tag="combr", bufs=2)
                nc.scalar.copy(combr_s, comb_s)
                # emb = comb @ (Wo2^T/L) as a single [1,D] row (the 1/L mean
                # factor is folded into wo2t on the host)
                ps_o = pp.tile([1, D], f32, tag="ps", name=f"pso_{b}")
                for ck in range(C2T):
                    mm(ps_o[0:1, :], combr_s[:, ck:ck + 1], wo2t_s[:, ck, :],
                       ck == 0, ck == C2T - 1)
                orow_s = vp.tile([1, D], f32, tag="orow", bufs=2)
                nc.scalar.copy(orow_s, ps_o)
                nc.sync.dma_start(out=out_ext[b:b + 1, :], in_=orow_s[0:1, :])

    _t1 = _time.time()
    nc.compile()
    print(f"[kernel] tile-trace+schedule {_t1 - _t0:.1f}s, "
          f"bacc compile {_time.time() - _t1:.1f}s", file=sys.stderr, flush=True)
    return nc


def get_nc():
    if "nc" not in _CACHE:
        _CACHE["nc"] = _build_nc()
    return _CACHE["nc"]


def make_in_maps(x, W1, Wo1, W2, Wo2):
    import ml_dtypes
    bf = ml_dtypes.bfloat16
    x = np.ascontiguousarray(np.asarray(x, dtype=np.float32))
    xT = np.ascontiguousarray(x.transpose(0, 2, 1)).astype(bf)
    xb = x.astype(bf)
    w1t = np.ascontiguousarray(np.asarray(W1, np.float32).T).astype(bf)
    wo1t = np.ascontiguousarray(np.asarray(Wo1, np.float32).T).astype(bf)
    w2t = np.ascontiguousarray(np.asarray(W2, np.float32).T).astype(bf)
    # 1/L mean-pooling factor folded into the stage-2 output projection
    wo2t = (np.ascontiguousarray(np.asarray(Wo2, np.float32).T)
            * (1.0 / L)).astype(bf)
    ident = np.eye(P, dtype=np.float32).astype(bf)
    onesc = np.ones((P, 1), dtype=np.float32).astype(bf)
    onesr = np.ones((1, P), dtype=np.float32).astype(bf)
    return [
        {"x": xb[c * B:(c + 1) * B], "xT": xT[c * B:(c + 1) * B],
         "w1t": w1t, "wo1t": wo1t, "w2t": w2t, "wo2t": wo2t,
         "ident": ident, "onesc": onesc, "onesr": onesr}
        for c in range(NCORES)
    ]


def run(x, W1, Wo1, W2, Wo2, trace=False, **kw):
    from concourse.bass_utils import run_bass_kernel_spmd
    nc = get_nc()
    in_maps = make_in_maps(x, W1, Wo1, W2, Wo2)
    res = run_bass_kernel_spmd(nc, in_maps, core_ids=list(range(NCORES)),
                               trace=trace, **kw)
    out = np.concatenate([res.results[c]["out"] for c in range(NCORES)], axis=0)
    return out.reshape(N_GLOBAL, D, 1, 1), res


def kernel(**inputs):
    out, _ = run(inputs["x"], inputs["W1"], inputs["Wo1"],
                 inputs["W2"], inputs["Wo2"])
    return out


# revision 35
# speedup vs baseline: 1.1045x; 1.1045x over previous
"""AttentionFuserV3 Trainium2 kernel: 8-core pure data parallel over batch.

Reference computation per batch item x_b [L=1024, D=512]:
  stage1: q = x W1^T; S = q x^T; A = softmax(S); mix = A x;
          h = tanh([mix, q] Wo1^T); h = h / max(||h||_2, eps)     (per row)
  stage2: c = [h, x]; q2 = c W2^T; S2 = q2 c^T; A2 = softmax(S2);
          mix2 = A2 c; o = [mix2, q2] Wo2^T; emb = mean_l(o)

Pooling algebra: emb = mean_l(o) is linear, so the full [L,2D] mix2 and
[L,D] output projection are never materialized.  Instead
  emb = [colsum(A2) c, colsum(q2)] (Wo2^T / L)
where colsum(A2)[m] = sum_l exp(S2[l,m]) / denom[l] is a cheap
multiply+reduce over the already-computed exp tiles.  This removes the
two largest matmul groups of stage 2.

Layout strategy ("T-space"): all big tensors are kept transposed in SBUF
(feature dim on partitions, sequence dim L on the free axis) so every
matmul contraction lands on the partition axis.  Softmax runs without
max-subtraction (|scores| < ~70, exp stays in range); the denominator is
accumulated with a ones-vector matmul and applied as a column broadcast
produced by a rank-1 matmul.

All matmul operands are bf16 (same 1 cycle/row PE speed as f32r, half
the SBUF/DMA); accumulation stays in f32 PSUM.  The halved SBUF lets
every per-batch tile be double-buffered, and the program is emitted as
an explicit software pipeline: stage 2 of batch b is interleaved with
stage 1 of batch b+1 at phase granularity (and stage-1 phases alternate
their two l-chunks) so the in-order PE queue always has independent
matmuls between a producer phase and its consumer.
"""

import sys

sys.path.insert(0, "/opt/trn_rl_repo")

import numpy as np

N_GLOBAL, L, D = 32, 1024, 512
NCORES = 8
B = N_GLOBAL // NCORES          # 4 batch items per core
P = 128
LC = 512                        # l-chunk (matmul moving free dim)
NLC = L // LC                   # 2
DT = D // P                     # 4
LT = L // P                     # 8
D2T = 2 * D // P                # 8
C2T = 4 * D // P                # 16

_CACHE = {}


def _build_nc():
    import concourse.bass as bass  # noqa: F401
    import concourse.mybir as mybir
    import concourse.tile as tile
    from concourse import bacc

    f32 = mybir.dt.float32
    bf16 = mybir.dt.bfloat16
    AF = mybir.ActivationFunctionType
    ALU = mybir.AluOpType

    nc = bacc.Bacc("TRN2", target_bir_lowering=False, debug=False,
                   num_devices=NCORES)

    x_ext = nc.declare_dram_parameter("x", [B, L, D], bf16, isOutput=False)
    xT_ext = nc.declare_dram_parameter("xT", [B, D, L], bf16, isOutput=False)
    w1t_ext = nc.declare_dram_parameter("w1t", [D, D], bf16, isOutput=False)
    wo1t_ext = nc.declare_dram_parameter("wo1t", [2 * D, D], bf16, isOutput=False)
    w2t_ext = nc.declare_dram_parameter("w2t", [2 * D, 2 * D], bf16, isOutput=False)
    wo2t_ext = nc.declare_dram_parameter("wo2t", [4 * D, D], bf16, isOutput=False)
    id_ext = nc.declare_dram_parameter("ident", [P, P], bf16, isOutput=False)
    onc_ext = nc.declare_dram_parameter("onesc", [P, 1], bf16, isOutput=False)
    onr_ext = nc.declare_dram_parameter("onesr", [1, P], bf16, isOutput=False)
    # DRAM bounce buffer for the pooled-mix row -> column transpose
    mscr_ext = nc.declare_dram_parameter("mscr", [B, 2 * D], f32, isOutput=True)
    out_ext = nc.declare_dram_parameter("out", [B, D], f32, isOutput=True)

    import time as _time
    _t0 = _time.time()
    with tile.TileContext(nc) as tc:
        with tc.tile_pool(name="wp", bufs=1) as wp, \
             tc.tile_pool(name="cp", bufs=1) as cp, \
             tc.tile_pool(name="xp", bufs=2) as xp, \
             tc.tile_pool(name="hp", bufs=2) as hp, \
             tc.tile_pool(name="tp", bufs=2) as tp, \
             tc.tile_pool(name="vp", bufs=2) as vp, \
             tc.tile_pool(name="ps", bufs=8, space="PSUM") as pp:

            # ---- weights (resident) ----
            w1t_s = wp.tile([P, DT, D], bf16, tag="w1t")
            nc.sync.dma_start(out=w1t_s, in_=w1t_ext.rearrange("(k p) e -> p k e", p=P))
            wo1t_s = wp.tile([P, D2T, D], bf16, tag="wo1t")
            nc.sync.dma_start(out=wo1t_s, in_=wo1t_ext.rearrange("(k p) e -> p k e", p=P))
            w2t_s = wp.tile([P, D2T, 2 * D], bf16, tag="w2t")
            nc.sync.dma_start(out=w2t_s, in_=w2t_ext.rearrange("(k p) e -> p k e", p=P))
            wo2t_s = wp.tile([P, C2T, D], bf16, tag="wo2t")
            nc.sync.dma_start(out=wo2t_s, in_=wo2t_ext.rearrange("(k p) e -> p k e", p=P))

            # ---- constants (DMA'd from host) ----
            ident_s = cp.tile([P, P], bf16, tag="ident")
            nc.sync.dma_start(out=ident_s, in_=id_ext[:, :])
            ones_s = cp.tile([P, 1], bf16, tag="ones")
            nc.sync.dma_start(out=ones_s, in_=onc_ext[:, :])
            onesr_s = cp.tile([1, P], bf16, tag="onesr")
            nc.sync.dma_start(out=onesr_s, in_=onr_ext[:, :])

            def mm(out, lhsT, rhs, first, last):
                nc.tensor.matmul(out, lhsT, rhs, start=first, stop=last)

            def bcast_recip(denom_ps, nm, clamp_eps=None):
                """[1,512] PSUM denominator -> [128,512] SBUF broadcast of its
                reciprocal (optionally sqrt+clamp first)."""
                rv = vp.tile([1, LC], bf16, tag="rv", bufs=2, name=f"rv_{nm}")
                with nc.allow_low_precision(reason="bf16 softmax scale"):
                    if clamp_eps is not None:
                        nv = vp.tile([1, LC], f32, tag="nv", bufs=2,
                                     name=f"nv_{nm}")
                        nc.scalar.sqrt(nv, denom_ps[0:1, :])
                        nc.vector.tensor_scalar_max(nv, nv, clamp_eps)
                        nc.vector.reciprocal(rv, nv)
                    else:
                        nc.vector.reciprocal(rv, denom_ps[0:1, :])
                ps_b = pp.tile([P, LC], f32, tag="ps", name=f"psb_{nm}")
                mm(ps_b, onesr_s, rv[0:1, :], True, True)
                bc = vp.tile([P, LC], f32, tag="bc", bufs=4, name=f"bc_{nm}")
                nc.scalar.copy(bc, ps_b)
                return bc

            # ---- per-batch tile state ----
            T = [dict() for _ in range(B)]

            def start_batch(b):
                t = T[b]
                t["xT"] = xp.tile([P, DT, L], bf16, tag="xT", name=f"xT_{b}")
                nc.sync.dma_start(out=t["xT"],
                                  in_=xT_ext[b].rearrange("(k p) l -> p k l", p=P))
                t["x"] = xp.tile([P, LT, D], bf16, tag="x", name=f"x_{b}")
                nc.sync.dma_start(out=t["x"],
                                  in_=x_ext[b].rearrange("(k p) d -> p k d", p=P))
                t["hTn"] = hp.tile([P, DT, L], bf16, tag="hTn", name=f"hTn_{b}")
                t["qT"] = [None] * NLC
                t["mixT"] = [None] * NLC
                t["expT"] = [None] * NLC
                t["hT"] = [None] * NLC
                t["q2T"] = [None] * NLC
                t["exp2"] = [None] * NLC
                t["bc1"] = [None] * NLC
                t["bc3"] = [None] * NLC

            # ================= stage 1 phases =================
            def ph1(b, lc):
                t = T[b]
                ls = slice(lc * LC, (lc + 1) * LC)
                qT = tp.tile([P, DT, LC], bf16, tag="qt", name=f"qT_{b}_{lc}")
                t["qT"][lc] = qT
                for et in range(DT):
                    ps = pp.tile([P, LC], f32, tag="ps", name=f"ps1_{b}_{lc}_{et}")
                    for dk in range(DT):
                        mm(ps, w1t_s[:, dk, et * P:(et + 1) * P],
                           t["xT"][:, dk, ls], dk == 0, dk == DT - 1)
                    nc.scalar.copy(qT[:, et, :], ps)

            def ph2(b, lc):
                t = T[b]
                expT = tp.tile([P, LT, LC], bf16, tag="exp", name=f"expT_{b}_{lc}")
                t["expT"][lc] = expT
                ps_d = pp.tile([P, LC], f32, tag="ps", name=f"psd1_{b}_{lc}")
                for mt in range(LT):
                    ps = pp.tile([P, LC], f32, tag="ps", name=f"ps2_{b}_{lc}_{mt}")
                    for ek in range(DT):
                        mm(ps, t["xT"][:, ek, mt * P:(mt + 1) * P],
                           t["qT"][lc][:, ek, :], ek == 0, ek == DT - 1)
                    nc.scalar.activation(expT[:, mt, :], ps, AF.Exp)
                    mm(ps_d[0:1, :], ones_s, expT[:, mt, :],
                       mt == 0, mt == LT - 1)
                t["bc1"][lc] = bcast_recip(ps_d, f"b1_{b}_{lc}")

            def ph3(b, lc):
                t = T[b]
                mixT = tp.tile([P, DT, LC], bf16, tag="mix", name=f"mixT_{b}_{lc}")
                t["mixT"][lc] = mixT
                ps_m = [pp.tile([P, LC], f32, tag="ps", name=f"psm_{b}_{lc}_{i}")
                        for i in range(DT)]
                for mk in range(LT):
                    for dt in range(DT):
                        mm(ps_m[dt], t["x"][:, mk, dt * P:(dt + 1) * P],
                           t["expT"][lc][:, mk, :], mk == 0, mk == LT - 1)
                for dt in range(DT):
                    nc.vector.tensor_mul(mixT[:, dt, :], ps_m[dt], t["bc1"][lc])

            def ph4(b, lc):
                t = T[b]
                hT = tp.tile([P, DT, LC], bf16, tag="ht", name=f"hT_{b}_{lc}")
                t["hT"][lc] = hT
                for ot in range(DT):
                    ps = pp.tile([P, LC], f32, tag="ps", name=f"ps4_{b}_{lc}_{ot}")
                    for ck in range(D2T):
                        rhs = (t["mixT"][lc][:, ck, :] if ck < DT
                               else t["qT"][lc][:, ck - DT, :])
                        mm(ps, wo1t_s[:, ck, ot * P:(ot + 1) * P],
                           rhs, ck == 0, ck == D2T - 1)
                    nc.scalar.activation(hT[:, ot, :], ps, AF.Tanh)

            def ph5(b, lc):
                t = T[b]
                ls = slice(lc * LC, (lc + 1) * LC)
                hsq = tp.tile([P, DT, LC], bf16, tag="hsq", name=f"hsq_{b}_{lc}")
                for dt in range(DT):
                    nc.scalar.activation(hsq[:, dt, :], t["hT"][lc][:, dt, :],
                                         AF.Square)
                ps_n = pp.tile([P, LC], f32, tag="ps", name=f"psn_{b}_{lc}")
                for dt in range(DT):
                    mm(ps_n[0:1, :], ones_s, hsq[:, dt, :], dt == 0, dt == DT - 1)
                bc2 = bcast_recip(ps_n, f"b2_{b}_{lc}", clamp_eps=1e-12)
                for dt in range(DT):
                    nc.vector.tensor_mul(t["hTn"][:, dt, ls], t["hT"][lc][:, dt, :],
                                         bc2)

            def ph6(b):
                # transpose hidden_norm back to natural layout
                t = T[b]
                t["hn_a"] = tp.tile([P, LT // 2, D], bf16, tag="hna",
                                    name=f"hna_{b}")
                t["hn_b"] = tp.tile([P, LT // 2, D], bf16, tag="hnb",
                                    name=f"hnb_{b}")
                for lt in range(LT):
                    dst = (t["hn_a"][:, lt, :] if lt < LT // 2
                           else t["hn_b"][:, lt - LT // 2, :])
                    for dt in range(DT):
                        ps_t = pp.tile([P, P], bf16, tag="ps",
                                       name=f"pst_{b}_{lt}_{dt}")
                        nc.tensor.transpose(
                            ps_t, t["hTn"][:, dt, lt * P:(lt + 1) * P], ident_s)
                        nc.scalar.copy(dst[:, dt * P:(dt + 1) * P], ps_t)

            # ================= stage 2 phases =================
            def c2T(t, k, fslice):
                """combined2T[d2,·] k-tile: [hTn; xT]"""
                return (t["hTn"][:, k, fslice] if k < DT
                        else t["xT"][:, k - DT, fslice])

            def ph7(b, lc):
                t = T[b]
                ls = slice(lc * LC, (lc + 1) * LC)
                if lc == 0:
                    t["q2r"] = vp.tile([P, D2T, NLC], f32, tag="q2r", bufs=2,
                                       name=f"q2r_{b}")
                q2T = tp.tile([P, D2T, LC], bf16, tag="q2", name=f"q2T_{b}_{lc}")
                t["q2T"][lc] = q2T
                for et in range(D2T):
                    ps = pp.tile([P, LC], f32, tag="ps", name=f"ps7_{b}_{lc}_{et}")
                    for dk in range(D2T):
                        mm(ps, w2t_s[:, dk, et * P:(et + 1) * P],
                           c2T(t, dk, ls), dk == 0, dk == D2T - 1)
                    nc.scalar.copy(q2T[:, et, :], ps)
                    nc.vector.tensor_reduce(t["q2r"][:, et, lc:lc + 1], ps,
                                            axis=mybir.AxisListType.X,
                                            op=ALU.add)

            def ph8(b, lc):
                t = T[b]
                if lc == 0:
                    t["a2p"] = vp.tile([P, LT, NLC], f32, tag="a2p", bufs=2,
                                       name=f"a2p_{b}")
                    t["scr"] = vp.tile([P, LC], f32, tag="scr", bufs=2,
                                       name=f"scr_{b}")
                exp2 = tp.tile([P, LT, LC], bf16, tag="exp", name=f"exp2_{b}_{lc}")
                t["exp2"][lc] = exp2
                ps_d = pp.tile([P, LC], f32, tag="ps", name=f"psd2_{b}_{lc}")
                for mt in range(LT):
                    ps = pp.tile([P, LC], f32, tag="ps", name=f"ps8_{b}_{lc}_{mt}")
                    for ek in range(D2T):
                        mm(ps, c2T(t, ek, slice(mt * P, (mt + 1) * P)),
                           t["q2T"][lc][:, ek, :], ek == 0, ek == D2T - 1)
                    nc.scalar.activation(exp2[:, mt, :], ps, AF.Exp)
                    mm(ps_d[0:1, :], ones_s, exp2[:, mt, :],
                       mt == 0, mt == LT - 1)
                bc3 = bcast_recip(ps_d, f"b3_{b}_{lc}")
                # A2 column sums a2p[m,lc] = sum_{l in chunk} exp2[m,l]/denom[l]
                for mt in range(LT):
                    nc.vector.tensor_mul(t["scr"], exp2[:, mt, :], bc3)
                    nc.vector.tensor_reduce(t["a2p"][:, mt, lc:lc + 1], t["scr"],
                                            axis=mybir.AxisListType.X,
                                            op=ALU.add)

            def epiA(b):
                # comb = [colsum(A2) @ c, colsum(q2)] as a [4D] column vector
                t = T[b]
                comb = vp.tile([P, C2T], f32, tag="comb", bufs=2, name=f"comb_{b}")
                t["comb"] = comb
                nc.vector.tensor_add(comb[:, D2T:C2T], t["q2r"][:, :, 0],
                                     t["q2r"][:, :, 1])
                a2s = vp.tile([P, LT], f32, tag="a2s", bufs=2, name=f"a2s_{b}")
                nc.vector.tensor_add(a2s, t["a2p"][:, :, 0], t["a2p"][:, :, 1])
                a2sr = vp.tile([P, LT], bf16, tag="a2sr", bufs=2, name=f"a2sr_{b}")
                nc.scalar.copy(a2sr, a2s)
                # colsum(A2) @ c over the natural-layout halves of c = [hn, x];
                # the [1,D] result rows become comb columns via a DRAM bounce
                for half in range(2):
                    ps_mx = pp.tile([1, D], f32, tag="ps", name=f"psmx_{b}_{half}")
                    for mk in range(LT):
                        if half == 0:
                            rhs = (t["hn_a"][:, mk, :] if mk < LT // 2
                                   else t["hn_b"][:, mk - LT // 2, :])
                        else:
                            rhs = t["x"][:, mk, :]
                        mm(ps_mx[0:1, :], a2sr[:, mk:mk + 1], rhs,
                           mk == 0, mk == LT - 1)
                    mrow = vp.tile([1, D], f32, tag="mrow", bufs=2,
                                   name=f"mrow_{b}_{half}")
                    nc.scalar.copy(mrow, ps_mx)
                    nc.sync.dma_start(
                        out=mscr_ext[b:b + 1, half * D:(half + 1) * D],
                        in_=mrow[0:1, :])
                nc.sync.dma_start(
                    out=comb[:, 0:D2T],
                    in_=mscr_ext[b].rearrange("(k p) -> p k", p=P))

            def epiB(b):
                # emb = comb @ (Wo2^T/L) as a single [1,D] row (1/L folded
                # into wo2t on the host)
                t = T[b]
                combr = vp.tile([P, C2T], bf16, tag="combr", bufs=2,
                                name=f"combr_{b}")
                nc.scalar.copy(combr, t["comb"])
                ps_o = pp.tile([1, D], f32, tag="ps", name=f"pso_{b}")
                for ck in range(C2T):
                    mm(ps_o[0:1, :], combr[:, ck:ck + 1], wo2t_s[:, ck, :],
                       ck == 0, ck == C2T - 1)
                orow = vp.tile([1, D], f32, tag="orow", bufs=2, name=f"orow_{b}")
                nc.scalar.copy(orow, ps_o)
                nc.sync.dma_start(out=out_ext[b:b + 1, :], in_=orow[0:1, :])

            # ================= emission schedule =================
            def S1(b):
                return [lambda lc=lc, f=f: f(b, lc)
                        for f in (ph1, ph2, ph3, ph4, ph5) for lc in range(NLC)
                        ] + [lambda: ph6(b)]

            def S2(b):
                return [lambda: ph7(b, 0), lambda: ph8(b, 0),
                        lambda: ph7(b, 1), lambda: ph8(b, 1),
                        lambda: epiA(b), lambda: epiB(b)]

            start_batch(0)
            for f in S1(0):
                f()
            for b in range(B):
                A = S2(b)
                if b + 1 < B:
                    start_batch(b + 1)
                    Bl = S1(b + 1)
                    # A0 B0 B1 A1 B2 B3 A2 B4 B5 A3 B6 B7 A4 B8 B9 B10 A5
                    A[0]()
                    Bl[0](); Bl[1]()
                    A[1]()
                    Bl[2](); Bl[3]()
                    A[2]()
                    Bl[4](); Bl[5]()
                    A[3]()
                    Bl[6](); Bl[7]()
                    A[4]()
                    Bl[8](); Bl[9](); Bl[10]()
                    A[5]()
                else:
                    for f in A:
                        f()

    _t1 = _time.time()
    nc.compile()
    print(f"[kernel] tile-trace+schedule {_t1 - _t0:.1f}s, "
          f"bacc compile {_time.time() - _t1:.1f}s", file=sys.stderr, flush=True)
    return nc


def get_nc():
    if "nc" not in _CACHE:
        _CACHE["nc"] = _build_nc()
    return _CACHE["nc"]


def make_in_maps(x, W1, Wo1, W2, Wo2):
    import ml_dtypes
    bf = ml_dtypes.bfloat16
    x = np.ascontiguousarray(np.asarray(x, dtype=np.float32))
    xT = np.ascontiguousarray(x.transpose(0, 2, 1)).astype(bf)
    xb = x.astype(bf)
    w1t = np.ascontiguousarray(np.asarray(W1, np.float32).T).astype(bf)
    wo1t = np.ascontiguousarray(np.asarray(Wo1, np.float32).T).astype(bf)
    w2t = np.ascontiguousarray(np.asarray(W2, np.float32).T).astype(bf)
    # 1/L mean-pooling factor folded into the stage-2 output projection
    wo2t = (np.ascontiguousarray(np.asarray(Wo2, np.float32).T)
            * (1.0 / L)).astype(bf)
    ident = np.eye(P, dtype=np.float32).astype(bf)
    onesc = np.ones((P, 1), dtype=np.float32).astype(bf)
    onesr = np.ones((1, P), dtype=np.float32).astype(bf)
    return [
        {"x": xb[c * B:(c + 1) * B], "xT": xT[c * B:(c + 1) * B],
         "w1t": w1t, "wo1t": wo1t, "w2t": w2t, "wo2t": wo2t,
         "ident": ident, "onesc": onesc, "onesr": onesr}
        for c in range(NCORES)
    ]


def run(x, W1, Wo1, W2, Wo2, trace=False, **kw):
    from concourse.bass_utils import run_bass_kernel_spmd
    nc = get_nc()
    in_maps = make_in_maps(x, W1, Wo1, W2, Wo2)
    res = run_bass_kernel_spmd(nc, in_maps, core_ids=list(range(NCORES)),
                               trace=trace, **kw)
    out = np.concatenate([res.results[c]["out"] for c in range(NCORES)], axis=0)
    return out.reshape(N_GLOBAL, D, 1, 1), res


def kernel(**inputs):
    out, _ = run(inputs["x"], inputs["W1"], inputs["Wo1"],
                 inputs["W2"], inputs["Wo2"])
    return out


# revision 45
# speedup vs baseline: 1.2700x; 1.1498x over previous
"""AttentionFuserV3 Trainium2 kernel: 8-core pure data parallel over batch.

Reference computation per batch item x_b [L=1024, D=512]:
  stage1: q = x W1^T; S = q x^T; A = softmax(S); mix = A x;
          h = tanh([mix, q] Wo1^T); h = h / max(||h||_2, eps)     (per row)
  stage2: c = [h, x]; q2 = c W2^T; S2 = q2 c^T; A2 = softmax(S2);
          mix2 = A2 c; o = [mix2, q2] Wo2^T; emb = mean_l(o)

Pooling algebra: emb = mean_l(o) is linear, so the full [L,2D] mix2 and
[L,D] output projection are never materialized.  Instead
  emb = [colsum(A2) c, colsum(q2)] (Wo2^T / L)
where colsum(A2)[m] = sum_l exp(S2[l,m]) / denom[l] is a cheap
multiply+reduce over the already-computed exp tiles.  This removes the
two largest matmul groups of stage 2.

Layout strategy ("T-space"): all big tensors are kept transposed in SBUF
(feature dim on partitions, sequence dim L on the free axis) so every
matmul contraction lands on the partition axis.  Softmax runs without
max-subtraction (|scores| < ~70, exp stays in range); the denominator is
accumulated with a ones-vector matmul and applied as a column broadcast
produced by a rank-1 matmul.

All matmul operands are bf16 (same 1 cycle/row PE speed as f32r, half
the SBUF/DMA); accumulation stays in f32 PSUM.  The halved SBUF lets
every per-batch tile be double-buffered, and the program is emitted as
an explicit software pipeline: stage 2 of batch b is interleaved with
stage 1 of batch b+1 at phase granularity (and stage-1 phases alternate
their two l-chunks) so the in-order PE queue always has independent
matmuls between a producer phase and its consumer.
"""

import sys

sys.path.insert(0, "/opt/trn_rl_repo")

import numpy as np

N_GLOBAL, L, D = 32, 1024, 512
NCORES = 8
B = N_GLOBAL // NCORES          # 4 batch items per core
P = 128
LC = 512                        # l-chunk (matmul moving free dim)
NLC = L // LC                   # 2
DT = D // P                     # 4
LT = L // P                     # 8
D2T = 2 * D // P                # 8
C2T = 4 * D // P                # 16

_CACHE = {}


def _build_nc():
    import concourse.bass as bass  # noqa: F401
    import concourse.mybir as mybir
    import concourse.tile as tile
    from concourse import bacc

    f32 = mybir.dt.float32
    bf16 = mybir.dt.bfloat16
    AF = mybir.ActivationFunctionType
    ALU = mybir.AluOpType

    nc = bacc.Bacc("TRN2", target_bir_lowering=False, debug=False,
                   num_devices=NCORES)

    x_ext = nc.declare_dram_parameter("x", [B, L, D], bf16, isOutput=False)
    xT_ext = nc.declare_dram_parameter("xT", [B, D, L], bf16, isOutput=False)
    w1t_ext = nc.declare_dram_parameter("w1t", [D, D], bf16, isOutput=False)
    wo1t_ext = nc.declare_dram_parameter("wo1t", [2 * D, D], bf16, isOutput=False)
    w2t_ext = nc.declare_dram_parameter("w2t", [2 * D, 2 * D], bf16, isOutput=False)
    wo2t_ext = nc.declare_dram_parameter("wo2t", [4 * D, D], bf16, isOutput=False)
    id_ext = nc.declare_dram_parameter("ident", [P, P], bf16, isOutput=False)
    onc_ext = nc.declare_dram_parameter("onesc", [P, 1], bf16, isOutput=False)
    onr_ext = nc.declare_dram_parameter("onesr", [1, P], bf16, isOutput=False)
    # DRAM bounce buffer for the pooled-mix row -> column transpose
    mscr_ext = nc.declare_dram_parameter("mscr", [B, 2 * D], f32, isOutput=True)
    out_ext = nc.declare_dram_parameter("out", [B, D], f32, isOutput=True)

    import time as _time
    _t0 = _time.time()
    with tile.TileContext(nc) as tc:
        with tc.tile_pool(name="wp", bufs=1) as wp, \
             tc.tile_pool(name="cp", bufs=1) as cp, \
             tc.tile_pool(name="xp", bufs=2) as xp, \
             tc.tile_pool(name="hp", bufs=2) as hp, \
             tc.tile_pool(name="tp", bufs=2) as tp, \
             tc.tile_pool(name="vp", bufs=2) as vp, \
             tc.tile_pool(name="ps", bufs=8, space="PSUM") as pp:

            # ---- per-batch tile state (input DMAs issued before the bulky
            # weight DMAs so ph1 of batch 0 can start early) ----
            T = [dict() for _ in range(B)]

            def start_batch(b):
                t = T[b]
                t["xT"] = xp.tile([P, DT, L], bf16, tag="xT", name=f"xT_{b}")
                nc.sync.dma_start(out=t["xT"],
                                  in_=xT_ext[b].rearrange("(k p) l -> p k l", p=P))
                t["x"] = xp.tile([P, LT, D], bf16, tag="x", name=f"x_{b}")
                nc.sync.dma_start(out=t["x"],
                                  in_=x_ext[b].rearrange("(k p) d -> p k d", p=P))
                t["hTn"] = hp.tile([P, DT, L], bf16, tag="hTn", name=f"hTn_{b}")
                t["qT"] = [None] * NLC
                t["mixT"] = [None] * NLC
                t["expT"] = [None] * NLC
                t["hT"] = [None] * NLC
                t["q2T"] = [None] * NLC
                t["exp2"] = [None] * NLC
                t["bc1"] = [None] * NLC
                t["a2sr"] = [None] * NLC

            start_batch(0)

            # ---- weights + constants (resident) ----
            w1t_s = wp.tile([P, DT, D], bf16, tag="w1t")
            nc.sync.dma_start(out=w1t_s, in_=w1t_ext.rearrange("(k p) e -> p k e", p=P))
            ones_s = cp.tile([P, 1], bf16, tag="ones")
            nc.sync.dma_start(out=ones_s, in_=onc_ext[:, :])
            onesr_s = cp.tile([1, P], bf16, tag="onesr")
            nc.sync.dma_start(out=onesr_s, in_=onr_ext[:, :])
            wo1t_s = wp.tile([P, D2T, D], bf16, tag="wo1t")
            nc.sync.dma_start(out=wo1t_s, in_=wo1t_ext.rearrange("(k p) e -> p k e", p=P))
            ident_s = cp.tile([P, P], bf16, tag="ident")
            nc.sync.dma_start(out=ident_s, in_=id_ext[:, :])
            w2t_s = wp.tile([P, D2T, 2 * D], bf16, tag="w2t")
            nc.sync.dma_start(out=w2t_s, in_=w2t_ext.rearrange("(k p) e -> p k e", p=P))
            wo2t_s = wp.tile([P, C2T, D], bf16, tag="wo2t")
            nc.sync.dma_start(out=wo2t_s, in_=wo2t_ext.rearrange("(k p) e -> p k e", p=P))

            def mm(out, lhsT, rhs, first, last):
                nc.tensor.matmul(out, lhsT, rhs, start=first, stop=last)

            def bcast_recip(denom_ps, nm, clamp_eps=None):
                """[1,512] PSUM denominator -> [128,512] SBUF broadcast of its
                reciprocal (optionally sqrt+clamp first)."""
                rv = vp.tile([1, LC], bf16, tag="rv", bufs=2, name=f"rv_{nm}")
                with nc.allow_low_precision(reason="bf16 softmax scale"):
                    if clamp_eps is not None:
                        nv = vp.tile([1, LC], f32, tag="nv", bufs=2,
                                     name=f"nv_{nm}")
                        nc.scalar.sqrt(nv, denom_ps[0:1, :])
                        nc.vector.tensor_scalar_max(nv, nv, clamp_eps)
                        nc.vector.reciprocal(rv, nv)
                    else:
                        nc.vector.reciprocal(rv, denom_ps[0:1, :])
                ps_b = pp.tile([P, LC], f32, tag="ps", name=f"psb_{nm}")
                mm(ps_b, onesr_s, rv[0:1, :], True, True)
                bc = vp.tile([P, LC], f32, tag="bc", bufs=4, name=f"bc_{nm}")
                nc.scalar.copy(bc, ps_b)
                return bc

            # ================= stage 1 phases =================
            def ph1(b, lc):
                t = T[b]
                ls = slice(lc * LC, (lc + 1) * LC)
                qT = tp.tile([P, DT, LC], bf16, tag="qt", name=f"qT_{b}_{lc}")
                t["qT"][lc] = qT
                for et in range(DT):
                    ps = pp.tile([P, LC], f32, tag="ps", name=f"ps1_{b}_{lc}_{et}")
                    for dk in range(DT):
                        mm(ps, w1t_s[:, dk, et * P:(et + 1) * P],
                           t["xT"][:, dk, ls], dk == 0, dk == DT - 1)
                    nc.scalar.copy(qT[:, et, :], ps)

            def ph2(b, lc):
                t = T[b]
                expT = tp.tile([P, LT, LC], bf16, tag="exp", name=f"expT_{b}_{lc}")
                t["expT"][lc] = expT
                ps_d = pp.tile([P, LC], f32, tag="ps", name=f"psd1_{b}_{lc}")
                for mt in range(LT):
                    ps = pp.tile([P, LC], f32, tag="ps", name=f"ps2_{b}_{lc}_{mt}")
                    for ek in range(DT):
                        mm(ps, t["xT"][:, ek, mt * P:(mt + 1) * P],
                           t["qT"][lc][:, ek, :], ek == 0, ek == DT - 1)
                    nc.scalar.activation(expT[:, mt, :], ps, AF.Exp)
                    mm(ps_d[0:1, :], ones_s, expT[:, mt, :],
                       mt == 0, mt == LT - 1)
                t["bc1"][lc] = bcast_recip(ps_d, f"b1_{b}_{lc}")

            def ph3(b, lc):
                t = T[b]
                mixT = tp.tile([P, DT, LC], bf16, tag="mix", name=f"mixT_{b}_{lc}")
                t["mixT"][lc] = mixT
                ps_m = [pp.tile([P, LC], f32, tag="ps", name=f"psm_{b}_{lc}_{i}")
                        for i in range(DT)]
                for mk in range(LT):
                    for dt in range(DT):
                        mm(ps_m[dt], t["x"][:, mk, dt * P:(dt + 1) * P],
                           t["expT"][lc][:, mk, :], mk == 0, mk == LT - 1)
                for dt in range(DT):
                    nc.vector.tensor_mul(mixT[:, dt, :], ps_m[dt], t["bc1"][lc])

            def ph4(b, lc):
                t = T[b]
                hT = tp.tile([P, DT, LC], bf16, tag="ht", name=f"hT_{b}_{lc}")
                t["hT"][lc] = hT
                for ot in range(DT):
                    ps = pp.tile([P, LC], f32, tag="ps", name=f"ps4_{b}_{lc}_{ot}")
                    for ck in range(D2T):
                        rhs = (t["mixT"][lc][:, ck, :] if ck < DT
                               else t["qT"][lc][:, ck - DT, :])
                        mm(ps, wo1t_s[:, ck, ot * P:(ot + 1) * P],
                           rhs, ck == 0, ck == D2T - 1)
                    nc.scalar.activation(hT[:, ot, :], ps, AF.Tanh)

            def ph5(b, lc):
                t = T[b]
                ls = slice(lc * LC, (lc + 1) * LC)
                hsq = tp.tile([P, DT, LC], bf16, tag="hsq", name=f"hsq_{b}_{lc}")
                for dt in range(DT):
                    nc.scalar.activation(hsq[:, dt, :], t["hT"][lc][:, dt, :],
                                         AF.Square)
                ps_n = pp.tile([P, LC], f32, tag="ps", name=f"psn_{b}_{lc}")
                for dt in range(DT):
                    mm(ps_n[0:1, :], ones_s, hsq[:, dt, :], dt == 0, dt == DT - 1)
                bc2 = bcast_recip(ps_n, f"b2_{b}_{lc}", clamp_eps=1e-12)
                for dt in range(DT):
                    nc.vector.tensor_mul(t["hTn"][:, dt, ls], t["hT"][lc][:, dt, :],
                                         bc2)

            def ph6(b):
                # transpose hidden_norm back to natural layout
                t = T[b]
                t["hn_a"] = tp.tile([P, LT // 2, D], bf16, tag="hna",
                                    name=f"hna_{b}")
                t["hn_b"] = tp.tile([P, LT // 2, D], bf16, tag="hnb",
                                    name=f"hnb_{b}")
                for lt in range(LT):
                    dst = (t["hn_a"][:, lt, :] if lt < LT // 2
                           else t["hn_b"][:, lt - LT // 2, :])
                    for dt in range(DT):
                        ps_t = pp.tile([P, P], bf16, tag="ps",
                                       name=f"pst_{b}_{lt}_{dt}")
                        nc.tensor.transpose(
                            ps_t, t["hTn"][:, dt, lt * P:(lt + 1) * P], ident_s)
                        nc.scalar.copy(dst[:, dt * P:(dt + 1) * P], ps_t)

            # ================= stage 2 phases =================
            def c2T(t, k, fslice):
                """combined2T[d2,·] k-tile: [hTn; xT]"""
                return (t["hTn"][:, k, fslice] if k < DT
                        else t["xT"][:, k - DT, fslice])

            def ph7(b, lc):
                t = T[b]
                ls = slice(lc * LC, (lc + 1) * LC)
                if lc == 0:
                    t["q2r"] = vp.tile([P, D2T, NLC], f32, tag="q2r", bufs=2,
                                       name=f"q2r_{b}")
                q2T = tp.tile([P, D2T, LC], bf16, tag="q2", name=f"q2T_{b}_{lc}")
                t["q2T"][lc] = q2T
                for et in range(D2T):
                    ps = pp.tile([P, LC], f32, tag="ps", name=f"ps7_{b}_{lc}_{et}")
                    for dk in range(D2T):
                        mm(ps, w2t_s[:, dk, et * P:(et + 1) * P],
                           c2T(t, dk, ls), dk == 0, dk == D2T - 1)
                    nc.scalar.copy(q2T[:, et, :], ps)
                    nc.vector.tensor_reduce(t["q2r"][:, et, lc:lc + 1], ps,
                                            axis=mybir.AxisListType.X,
                                            op=ALU.add)

            def ph8(b, lc):
                t = T[b]
                if lc == 0:
                    t["a2p"] = vp.tile([P, LT, NLC], f32, tag="a2p", bufs=2,
                                       name=f"a2p_{b}")
                    t["scr"] = vp.tile([P, LC], f32, tag="scr", bufs=2,
                                       name=f"scr_{b}")
                exp2 = tp.tile([P, LT, LC], bf16, tag="exp", name=f"exp2_{b}_{lc}")
                t["exp2"][lc] = exp2
                ps_d = pp.tile([P, LC], f32, tag="ps", name=f"psd2_{b}_{lc}")
                for mt in range(LT):
                    ps = pp.tile([P, LC], f32, tag="ps", name=f"ps8_{b}_{lc}_{mt}")
                    for ek in range(D2T):
                        mm(ps, c2T(t, ek, slice(mt * P, (mt + 1) * P)),
                           t["q2T"][lc][:, ek, :], ek == 0, ek == D2T - 1)
                    nc.scalar.activation(exp2[:, mt, :], ps, AF.Exp)
                    mm(ps_d[0:1, :], ones_s, exp2[:, mt, :],
                       mt == 0, mt == LT - 1)
                bc3 = bcast_recip(ps_d, f"b3_{b}_{lc}")
                # A2 column sums a2p[m,lc] = sum_{l in chunk} exp2[m,l]/denom[l]
                # (on GpSimd: keeps the DVE queue short for the hTn/mixT muls
                # that gate the PE)
                for mt in range(LT):
                    nc.gpsimd.tensor_mul(t["scr"], exp2[:, mt, :], bc3)
                    nc.vector.tensor_reduce(t["a2p"][:, mt, lc:lc + 1], t["scr"],
                                            axis=mybir.AxisListType.X,
                                            op=ALU.add)
                # chunk-partial colsum(A2) weights for the pooled mix
                a2sr = vp.tile([P, LT], bf16, tag="a2sr", bufs=4,
                               name=f"a2sr_{b}_{lc}")
                nc.scalar.copy(a2sr, t["a2p"][:, :, lc])
                t["a2sr"][lc] = a2sr

            def _c_half(t, half, mk):
                if half == 0:
                    return (t["hn_a"][:, mk, :] if mk < LT // 2
                            else t["hn_b"][:, mk - LT // 2, :])
                return t["x"][:, mk, :]

            def psmx0(b):
                # pooled mix, chunk-0 partial: rows colsum_0(A2) @ c halves
                t = T[b]
                t["mpart"] = []
                for half in range(2):
                    ps1 = pp.tile([1, D], f32, tag="ps", name=f"psmx0_{b}_{half}")
                    for mk in range(LT):
                        mm(ps1[0:1, :], t["a2sr"][0][:, mk:mk + 1],
                           _c_half(t, half, mk), mk == 0, mk == LT - 1)
                    mp = vp.tile([1, D], f32, tag="mpart", bufs=2,
                                 name=f"mpart_{b}_{half}")
                    nc.scalar.copy(mp, ps1)
                    t["mpart"].append(mp)

            def epi_mid(b):
                # chunk-1 partial + combine; rows become comb columns via a
                # DRAM bounce (cross-partition move)
                t = T[b]
                for half in range(2):
                    ps2 = pp.tile([1, D], f32, tag="ps", name=f"psmx1_{b}_{half}")
                    for mk in range(LT):
                        mm(ps2[0:1, :], t["a2sr"][1][:, mk:mk + 1],
                           _c_half(t, half, mk), mk == 0, mk == LT - 1)
                    mrow = vp.tile([1, D], f32, tag="mrow", bufs=2,
                                   name=f"mrow_{b}_{half}")
                    nc.vector.tensor_add(mrow, t["mpart"][half], ps2[0:1, :])
                    nc.sync.dma_start(
                        out=mscr_ext[b:b + 1, half * D:(half + 1) * D],
                        in_=mrow[0:1, :])
                comb = vp.tile([P, C2T], f32, tag="comb", bufs=2, name=f"comb_{b}")
                t["comb"] = comb
                nc.vector.tensor_add(comb[:, D2T:C2T], t["q2r"][:, :, 0],
                                     t["q2r"][:, :, 1])
                nc.sync.dma_start(
                    out=comb[:, 0:D2T],
                    in_=mscr_ext[b].rearrange("(k p) -> p k", p=P))

            def epiB(b):
                # emb = comb @ (Wo2^T/L) as a single [1,D] row (1/L folded
                # into wo2t on the host)
                t = T[b]
                combr = vp.tile([P, C2T], bf16, tag="combr", bufs=2,
                                name=f"combr_{b}")
                nc.scalar.copy(combr, t["comb"])
                ps_o = pp.tile([1, D], f32, tag="ps", name=f"pso_{b}")
                for ck in range(C2T):
                    mm(ps_o[0:1, :], combr[:, ck:ck + 1], wo2t_s[:, ck, :],
                       ck == 0, ck == C2T - 1)
                orow = vp.tile([1, D], f32, tag="orow", bufs=2, name=f"orow_{b}")
                nc.scalar.copy(orow, ps_o)
                nc.sync.dma_start(out=out_ext[b:b + 1, :], in_=orow[0:1, :])

            # ================= emission schedule =================
            def S1(b):
                return [lambda lc=lc, f=f: f(b, lc)
                        for f in (ph1, ph2, ph3, ph4, ph5) for lc in range(NLC)
                        ] + [lambda: ph6(b)]

            def S2(b):
                return [lambda: ph7(b, 0), lambda: ph8(b, 0),
                        lambda: (psmx0(b), ph7(b, 1)), lambda: ph8(b, 1),
                        lambda: epi_mid(b), lambda: epiB(b)]

            for f in S1(0):
                f()
            for b in range(B):
                A = S2(b)
                if b + 1 < B:
                    start_batch(b + 1)
                    Bl = S1(b + 1)
                    # A0 B0 B1 A1 B2 B3 A2 B4 B5 A3 B6 B7 A4 B8 B9 B10 A5
                    A[0]()
                    Bl[0](); Bl[1]()
                    A[1]()
                    Bl[2](); Bl[3]()
                    A[2]()
                    Bl[4](); Bl[5]()
                    A[3]()
                    Bl[6](); Bl[7]()
                    A[4]()
                    Bl[8](); Bl[9](); Bl[10]()
                    A[5]()
                else:
                    for f in A:
                        f()

    _t1 = _time.time()
    nc.compile()
    print(f"[kernel] tile-trace+schedule {_t1 - _t0:.1f}s, "
          f"bacc compile {_time.time() - _t1:.1f}s", file=sys.stderr, flush=True)
    return nc


def get_nc():
    if "nc" not in _CACHE:
        _CACHE["nc"] = _build_nc()
    return _CACHE["nc"]


def make_in_maps(x, W1, Wo1, W2, Wo2):
    import ml_dtypes
    bf = ml_dtypes.bfloat16
    x = np.ascontiguousarray(np.asarray(x, dtype=np.float32))
    xT = np.ascontiguousarray(x.transpose(0, 2, 1)).astype(bf)
    xb = x.astype(bf)
    w1t = np.ascontiguousarray(np.asarray(W1, np.float32).T).astype(bf)
    wo1t = np.ascontiguousarray(np.asarray(Wo1, np.float32).T).astype(bf)
    w2t = np.ascontiguousarray(np.asarray(W2, np.float32).T).astype(bf)
    # 1/L mean-pooling factor folded into the stage-2 output projection
    wo2t = (np.ascontiguousarray(np.asarray(Wo2, np.float32).T)
            * (1.0 / L)).astype(bf)
    ident = np.eye(P, dtype=np.float32).astype(bf)
    onesc = np.ones((P, 1), dtype=np.float32).astype(bf)
    onesr = np.ones((1, P), dtype=np.float32).astype(bf)
    return [
        {"x": xb[c * B:(c + 1) * B], "xT": xT[c * B:(c + 1) * B],
         "w1t": w1t, "wo1t": wo1t, "w2t": w2t, "wo2t": wo2t,
         "ident": ident, "onesc": onesc, "onesr": onesr}
        for c in range(NCORES)
    ]


def run(x, W1, Wo1, W2, Wo2, trace=False, **kw):
    from concourse.bass_utils import run_bass_kernel_spmd
    nc = get_nc()
    in_maps = make_in_maps(x, W1, Wo1, W2, Wo2)
    res = run_bass_kernel_spmd(nc, in_maps, core_ids=list(range(NCORES)),
                               trace=trace, **kw)
    out = np.concatenate([res.results[c]["out"] for c in range(NCORES)], axis=0)
    return out.reshape(N_GLOBAL, D, 1, 1), res


def kernel(**inputs):
    out, _ = run(inputs["x"], inputs["W1"], inputs["Wo1"],
                 inputs["W2"], inputs["Wo2"])
    return out


# revision 53
# speedup vs baseline: 1.5108x; 1.1896x over previous
"""AttentionFuserV3 Trainium2 kernel: 8-core pure data parallel over batch.

Reference computation per batch item x_b [L=1024, D=512]:
  stage1: q = x W1^T; S = q x^T; A = softmax(S); mix = A x;
          h = tanh([mix, q] Wo1^T); h = h / max(||h||_2, eps)     (per row)
  stage2: c = [h, x]; q2 = c W2^T; S2 = q2 c^T; A2 = softmax(S2);
          mix2 = A2 c; o = [mix2, q2] Wo2^T; emb = mean_l(o)

Pooling algebra: emb = mean_l(o) is linear, so the full [L,2D] mix2 and
[L,D] output projection are never materialized.  Instead
  emb = [colsum(A2) c, colsum(q2)] (Wo2^T / L)
where colsum(A2)[m] = sum_l exp(S2[l,m]) / denom[l] is a cheap
multiply+reduce over the already-computed exp tiles.  This removes the
two largest matmul groups of stage 2.

Layout strategy ("T-space"): all big tensors are kept transposed in SBUF
(feature dim on partitions, sequence dim L on the free axis) so every
matmul contraction lands on the partition axis.  Softmax runs without
max-subtraction (|scores| < ~70, exp stays in range); the denominator is
accumulated with a ones-vector matmul and applied as a column broadcast
produced by a rank-1 matmul.

All matmul operands are bf16 (same 1 cycle/row PE speed as f32r, half
the SBUF/DMA); accumulation stays in f32 PSUM.  The halved SBUF lets
every per-batch tile be double-buffered, and the program is emitted as
an explicit software pipeline: stage 2 of batch b is interleaved with
stage 1 of batch b+1 at phase granularity (and stage-1 phases alternate
their two l-chunks) so the in-order PE queue always has independent
matmuls between a producer phase and its consumer.
"""

import sys

sys.path.insert(0, "/opt/trn_rl_repo")

import numpy as np

N_GLOBAL, L, D = 32, 1024, 512
NCORES = 8
B = N_GLOBAL // NCORES          # 4 batch items per core
P = 128
LC = 512                        # l-chunk (matmul moving free dim)
NLC = L // LC                   # 2
DT = D // P                     # 4
LT = L // P                     # 8
D2T = 2 * D // P                # 8
C2T = 4 * D // P                # 16

_CACHE = {}


def _build_nc():
    import concourse.bass as bass  # noqa: F401
    import concourse.mybir as mybir
    import concourse.tile as tile
    from concourse import bacc

    f32 = mybir.dt.float32
    bf16 = mybir.dt.bfloat16
    AF = mybir.ActivationFunctionType
    ALU = mybir.AluOpType

    nc = bacc.Bacc("TRN2", target_bir_lowering=False, debug=False,
                   num_devices=NCORES)

    x_ext = nc.declare_dram_parameter("x", [B, L, D], bf16, isOutput=False)
    xT_ext = nc.declare_dram_parameter("xT", [B, D, L], bf16, isOutput=False)
    w1t_ext = nc.declare_dram_parameter("w1t", [D, D], bf16, isOutput=False)
    wo1t_ext = nc.declare_dram_parameter("wo1t", [2 * D, D], bf16, isOutput=False)
    w2t_ext = nc.declare_dram_parameter("w2t", [2 * D, 2 * D], bf16, isOutput=False)
    wo2t_ext = nc.declare_dram_parameter("wo2t", [4 * D, D], bf16, isOutput=False)
    id_ext = nc.declare_dram_parameter("ident", [P, P], bf16, isOutput=False)
    onc_ext = nc.declare_dram_parameter("onesc", [P, 1], bf16, isOutput=False)
    onr_ext = nc.declare_dram_parameter("onesr", [1, P], bf16, isOutput=False)
    # DRAM bounce buffer for the pooled-mix row -> column transpose
    mscr_ext = nc.declare_dram_parameter("mscr", [B, 2 * D], f32, isOutput=True)
    out_ext = nc.declare_dram_parameter("out", [B, D], f32, isOutput=True)

    import time as _time
    _t0 = _time.time()
    with tile.TileContext(nc) as tc:
        with tc.tile_pool(name="wp", bufs=1) as wp, \
             tc.tile_pool(name="cp", bufs=1) as cp, \
             tc.tile_pool(name="xp", bufs=2) as xp, \
             tc.tile_pool(name="hp", bufs=2) as hp, \
             tc.tile_pool(name="tp", bufs=2) as tp, \
             tc.tile_pool(name="vp", bufs=2) as vp, \
             tc.tile_pool(name="ps", bufs=8, space="PSUM") as pp:

            # ---- per-batch tile state (input DMAs issued before the bulky
            # weight DMAs so ph1 of batch 0 can start early) ----
            T = [dict() for _ in range(B)]

            def start_batch(b):
                t = T[b]
                t["xT"] = xp.tile([P, DT, L], bf16, tag="xT", name=f"xT_{b}")
                nc.sync.dma_start(out=t["xT"],
                                  in_=xT_ext[b].rearrange("(k p) l -> p k l", p=P))
                t["x"] = xp.tile([P, LT, D], bf16, tag="x", name=f"x_{b}")
                nc.sync.dma_start(out=t["x"],
                                  in_=x_ext[b].rearrange("(k p) d -> p k d", p=P))
                t["hTn"] = hp.tile([P, DT, L], bf16, tag="hTn", name=f"hTn_{b}")
                t["qT"] = [None] * NLC
                t["mixT"] = [None] * NLC
                t["expT"] = [None] * NLC
                t["hT"] = [None] * NLC
                t["q2T"] = [None] * NLC
                t["exp2"] = [None] * NLC
                t["rv1"] = [None] * NLC
                t["rv2"] = [None] * NLC
                t["rv3"] = [None] * NLC
                t["a2sr"] = [None] * NLC

            start_batch(0)

            # ---- weights + constants (resident) ----
            w1t_s = wp.tile([P, DT, D], bf16, tag="w1t")
            nc.sync.dma_start(out=w1t_s, in_=w1t_ext.rearrange("(k p) e -> p k e", p=P))
            ones_s = cp.tile([P, 1], bf16, tag="ones")
            nc.sync.dma_start(out=ones_s, in_=onc_ext[:, :])
            onesr_s = cp.tile([1, P], bf16, tag="onesr")
            nc.sync.dma_start(out=onesr_s, in_=onr_ext[:, :])
            wo1t_s = wp.tile([P, D2T, D], bf16, tag="wo1t")
            nc.sync.dma_start(out=wo1t_s, in_=wo1t_ext.rearrange("(k p) e -> p k e", p=P))
            ident_s = cp.tile([P, P], bf16, tag="ident")
            nc.sync.dma_start(out=ident_s, in_=id_ext[:, :])
            w2t_s = wp.tile([P, D2T, 2 * D], bf16, tag="w2t")
            nc.sync.dma_start(out=w2t_s, in_=w2t_ext.rearrange("(k p) e -> p k e", p=P))
            wo2t_s = wp.tile([P, C2T, D], bf16, tag="wo2t")
            nc.sync.dma_start(out=wo2t_s, in_=wo2t_ext.rearrange("(k p) e -> p k e", p=P))

            def mm(out, lhsT, rhs, first, last):
                nc.tensor.matmul(out, lhsT, rhs, start=first, stop=last)

            def recip_part(denom_ps, nm, clamp_eps=None):
                """[1,512] PSUM denominator -> [1,512] SBUF reciprocal
                (optionally sqrt+clamp first)."""
                rv = vp.tile([1, LC], bf16, tag="rv", bufs=3, name=f"rv_{nm}")
                with nc.allow_low_precision(reason="bf16 softmax scale"):
                    if clamp_eps is not None:
                        nv = vp.tile([1, LC], f32, tag="nv", bufs=1,
                                     name=f"nv_{nm}")
                        nc.scalar.sqrt(nv, denom_ps[0:1, :])
                        nc.vector.tensor_scalar_max(nv, nv, clamp_eps)
                        nc.vector.reciprocal(rv, nv)
                    else:
                        nc.vector.reciprocal(rv, denom_ps[0:1, :])
                return rv

            def bcast_part(rv, nm):
                """[1,512] reciprocal -> [128,512] broadcast via rank-1 matmul.
                Emitted a block after recip_part so the PE never waits on the
                DVE reciprocal."""
                ps_b = pp.tile([P, LC], f32, tag="ps", name=f"psb_{nm}")
                mm(ps_b, onesr_s, rv[0:1, :], True, True)
                bc = vp.tile([P, LC], f32, tag="bc", bufs=3, name=f"bc_{nm}")
                nc.scalar.copy(bc, ps_b)
                return bc

            # ================= stage 1 phases =================
            def ph1(b, lc):
                t = T[b]
                ls = slice(lc * LC, (lc + 1) * LC)
                qT = tp.tile([P, DT, LC], bf16, tag="qt", name=f"qT_{b}_{lc}")
                t["qT"][lc] = qT
                for et in range(DT):
                    ps = pp.tile([P, LC], f32, tag="ps", name=f"ps1_{b}_{lc}_{et}")
                    for dk in range(DT):
                        mm(ps, w1t_s[:, dk, et * P:(et + 1) * P],
                           t["xT"][:, dk, ls], dk == 0, dk == DT - 1)
                    nc.scalar.copy(qT[:, et, :], ps)

            def ph2(b, lc):
                t = T[b]
                expT = tp.tile([P, LT, LC], bf16, tag="exp", bufs=3,
                               name=f"expT_{b}_{lc}")
                t["expT"][lc] = expT
                ps_d = pp.tile([P, LC], f32, tag="ps", name=f"psd1_{b}_{lc}")
                for mt in range(LT):
                    ps = pp.tile([P, LC], f32, tag="ps", name=f"ps2_{b}_{lc}_{mt}")
                    for ek in range(DT):
                        mm(ps, t["xT"][:, ek, mt * P:(mt + 1) * P],
                           t["qT"][lc][:, ek, :], ek == 0, ek == DT - 1)
                    nc.scalar.activation(expT[:, mt, :], ps, AF.Exp)
                    mm(ps_d[0:1, :], ones_s, expT[:, mt, :],
                       mt == 0, mt == LT - 1)
                t["rv1"][lc] = recip_part(ps_d, f"b1_{b}_{lc}")

            def ph3(b, lc):
                t = T[b]
                bc1 = bcast_part(t["rv1"][lc], f"b1_{b}_{lc}")
                mixT = tp.tile([P, DT, LC], bf16, tag="mix", name=f"mixT_{b}_{lc}")
                t["mixT"][lc] = mixT
                ps_m = [pp.tile([P, LC], f32, tag="ps", name=f"psm_{b}_{lc}_{i}")
                        for i in range(DT)]
                for mk in range(LT):
                    for dt in range(DT):
                        mm(ps_m[dt], t["x"][:, mk, dt * P:(dt + 1) * P],
                           t["expT"][lc][:, mk, :], mk == 0, mk == LT - 1)
                for dt in range(DT):
                    nc.vector.tensor_mul(mixT[:, dt, :], ps_m[dt], bc1)

            def ph4(b, lc):
                t = T[b]
                hT = tp.tile([P, DT, LC], bf16, tag="ht", name=f"hT_{b}_{lc}")
                t["hT"][lc] = hT
                for ot in range(DT):
                    ps = pp.tile([P, LC], f32, tag="ps", name=f"ps4_{b}_{lc}_{ot}")
                    for ck in range(D2T):
                        rhs = (t["mixT"][lc][:, ck, :] if ck < DT
                               else t["qT"][lc][:, ck - DT, :])
                        mm(ps, wo1t_s[:, ck, ot * P:(ot + 1) * P],
                           rhs, ck == 0, ck == D2T - 1)
                    nc.scalar.activation(hT[:, ot, :], ps, AF.Tanh)

            def ph5a(b, lc):
                t = T[b]
                hsq = tp.tile([P, DT, LC], bf16, tag="hsq", name=f"hsq_{b}_{lc}")
                for dt in range(DT):
                    nc.scalar.activation(hsq[:, dt, :], t["hT"][lc][:, dt, :],
                                         AF.Square)
                ps_n = pp.tile([P, LC], f32, tag="ps", name=f"psn_{b}_{lc}")
                for dt in range(DT):
                    mm(ps_n[0:1, :], ones_s, hsq[:, dt, :], dt == 0, dt == DT - 1)
                t["rv2"][lc] = recip_part(ps_n, f"b2_{b}_{lc}", clamp_eps=1e-12)

            def ph5b(b, lc):
                t = T[b]
                ls = slice(lc * LC, (lc + 1) * LC)
                bc2 = bcast_part(t["rv2"][lc], f"b2_{b}_{lc}")
                for dt in range(DT):
                    nc.vector.tensor_mul(t["hTn"][:, dt, ls], t["hT"][lc][:, dt, :],
                                         bc2)

            def ph6(b, half):
                # transpose hidden_norm back to natural layout (half = l-chunk)
                t = T[b]
                if half == 0:
                    t["hn_a"] = tp.tile([P, LT // 2, D], bf16, tag="hna",
                                        name=f"hna_{b}")
                    t["hn_b"] = tp.tile([P, LT // 2, D], bf16, tag="hnb",
                                        name=f"hnb_{b}")
                for lt in range(half * LT // 2, (half + 1) * LT // 2):
                    dst = (t["hn_a"][:, lt, :] if lt < LT // 2
                           else t["hn_b"][:, lt - LT // 2, :])
                    for dt in range(DT):
                        ps_t = pp.tile([P, P], bf16, tag="ps",
                                       name=f"pst_{b}_{lt}_{dt}")
                        nc.tensor.transpose(
                            ps_t, t["hTn"][:, dt, lt * P:(lt + 1) * P], ident_s)
                        nc.scalar.copy(dst[:, dt * P:(dt + 1) * P], ps_t)

            # ================= stage 2 phases =================
            def c2T(t, k, fslice):
                """combined2T[d2,·] k-tile: [hTn; xT]"""
                return (t["hTn"][:, k, fslice] if k < DT
                        else t["xT"][:, k - DT, fslice])

            def ph7(b, lc):
                t = T[b]
                ls = slice(lc * LC, (lc + 1) * LC)
                if lc == 0:
                    t["q2r"] = vp.tile([P, D2T, NLC], f32, tag="q2r", bufs=2,
                                       name=f"q2r_{b}")
                q2T = tp.tile([P, D2T, LC], bf16, tag="q2", name=f"q2T_{b}_{lc}")
                t["q2T"][lc] = q2T
                for et in range(D2T):
                    ps = pp.tile([P, LC], f32, tag="ps", name=f"ps7_{b}_{lc}_{et}")
                    for dk in range(D2T):
                        mm(ps, w2t_s[:, dk, et * P:(et + 1) * P],
                           c2T(t, dk, ls), dk == 0, dk == D2T - 1)
                    nc.scalar.copy(q2T[:, et, :], ps)
                    nc.vector.tensor_reduce(t["q2r"][:, et, lc:lc + 1], ps,
                                            axis=mybir.AxisListType.X,
                                            op=ALU.add)

            def ph8a(b, lc):
                t = T[b]
                if lc == 0:
                    t["a2p"] = vp.tile([P, LT, NLC], f32, tag="a2p", bufs=2,
                                       name=f"a2p_{b}")
                    t["scr"] = vp.tile([P, LC], f32, tag="scr", bufs=1,
                                       name=f"scr_{b}")
                exp2 = tp.tile([P, LT, LC], bf16, tag="exp", bufs=3,
                                name=f"exp2_{b}_{lc}")
                t["exp2"][lc] = exp2
                ps_d = pp.tile([P, LC], f32, tag="ps", name=f"psd2_{b}_{lc}")
                for mt in range(LT):
                    ps = pp.tile([P, LC], f32, tag="ps", name=f"ps8_{b}_{lc}_{mt}")
                    for ek in range(D2T):
                        mm(ps, c2T(t, ek, slice(mt * P, (mt + 1) * P)),
                           t["q2T"][lc][:, ek, :], ek == 0, ek == D2T - 1)
                    nc.scalar.activation(exp2[:, mt, :], ps, AF.Exp)
                    mm(ps_d[0:1, :], ones_s, exp2[:, mt, :],
                       mt == 0, mt == LT - 1)
                t["rv3"][lc] = recip_part(ps_d, f"b3_{b}_{lc}")

            def ph8b(b, lc):
                # A2 column sums a2p[m,lc] = sum_{l in chunk} exp2[m,l]/denom[l]
                # via fused multiply+reduce on the DVE
                t = T[b]
                bc3 = bcast_part(t["rv3"][lc], f"b3_{b}_{lc}")
                for mt in range(LT):
                    nc.vector.affine_mul_reduce(
                        out=t["scr"], accum_out=t["a2p"][:, mt, lc:lc + 1],
                        in0=t["exp2"][lc][:, mt, :], in1=bc3,
                        scale=1.0, bias=0.0)
                # chunk-partial colsum(A2) weights for the pooled mix
                a2sr = vp.tile([P, LT], bf16, tag="a2sr", bufs=4,
                               name=f"a2sr_{b}_{lc}")
                nc.scalar.copy(a2sr, t["a2p"][:, :, lc])
                t["a2sr"][lc] = a2sr

            def _c_half(t, half, mk):
                if half == 0:
                    return (t["hn_a"][:, mk, :] if mk < LT // 2
                            else t["hn_b"][:, mk - LT // 2, :])
                return t["x"][:, mk, :]

            def psmx0(b):
                # pooled mix, chunk-0 partial: rows colsum_0(A2) @ c halves
                t = T[b]
                t["mpart"] = []
                for half in range(2):
                    ps1 = pp.tile([1, D], f32, tag="ps", name=f"psmx0_{b}_{half}")
                    for mk in range(LT):
                        mm(ps1[0:1, :], t["a2sr"][0][:, mk:mk + 1],
                           _c_half(t, half, mk), mk == 0, mk == LT - 1)
                    mp = vp.tile([1, D], f32, tag="mpart", bufs=2,
                                 name=f"mpart_{b}_{half}")
                    nc.scalar.copy(mp, ps1)
                    t["mpart"].append(mp)

            def epi_mid(b):
                # chunk-1 partial + combine; rows become comb columns via a
                # DRAM bounce (cross-partition move)
                t = T[b]
                for half in range(2):
                    ps2 = pp.tile([1, D], f32, tag="ps", name=f"psmx1_{b}_{half}")
                    for mk in range(LT):
                        mm(ps2[0:1, :], t["a2sr"][1][:, mk:mk + 1],
                           _c_half(t, half, mk), mk == 0, mk == LT - 1)
                    mrow = vp.tile([1, D], f32, tag="mrow", bufs=2,
                                   name=f"mrow_{b}_{half}")
                    nc.vector.tensor_add(mrow, t["mpart"][half], ps2[0:1, :])
                    nc.sync.dma_start(
                        out=mscr_ext[b:b + 1, half * D:(half + 1) * D],
                        in_=mrow[0:1, :])
                comb = vp.tile([P, C2T], f32, tag="comb", bufs=2, name=f"comb_{b}")
                t["comb"] = comb
                nc.vector.tensor_add(comb[:, D2T:C2T], t["q2r"][:, :, 0],
                                     t["q2r"][:, :, 1])
                nc.sync.dma_start(
                    out=comb[:, 0:D2T],
                    in_=mscr_ext[b].rearrange("(k p) -> p k", p=P))

            def epiB(b):
                # emb = comb @ (Wo2^T/L) as a single [1,D] row (1/L folded
                # into wo2t on the host)
                t = T[b]
                combr = vp.tile([P, C2T], bf16, tag="combr", bufs=2,
                                name=f"combr_{b}")
                nc.scalar.copy(combr, t["comb"])
                ps_o = pp.tile([1, D], f32, tag="ps", name=f"pso_{b}")
                for ck in range(C2T):
                    mm(ps_o[0:1, :], combr[:, ck:ck + 1], wo2t_s[:, ck, :],
                       ck == 0, ck == C2T - 1)
                orow = vp.tile([1, D], f32, tag="orow", bufs=2, name=f"orow_{b}")
                nc.scalar.copy(orow, ps_o)
                nc.sync.dma_start(out=out_ext[b:b + 1, :], in_=orow[0:1, :])

            # ================= emission schedule =================
            # Software pipeline: stage 2 of batch b (A-blocks) interleaved
            # with stage 1 of batch b+1 (B-blocks); epiB(b) is deferred into
            # iteration b+1 so the DRAM bounce is in flight under ph7.
            def S1(b):
                return [lambda lc=lc, f=f: f(b, lc)
                        for f in (ph1, ph2, ph3, ph4) for lc in range(NLC)
                        ] + [lambda: ph5a(b, 0), lambda: ph5a(b, 1),
                             lambda: ph5b(b, 0), lambda: ph6(b, 0),
                             lambda: ph5b(b, 1), lambda: ph6(b, 1)]

            def S2(b):
                return [lambda: ph7(b, 0),
                        lambda: ph8a(b, 0),
                        lambda: (ph8b(b, 0), ph7(b, 1), psmx0(b)),
                        lambda: ph8a(b, 1),
                        lambda: ph8b(b, 1),
                        lambda: epi_mid(b)]

            for f in S1(0):
                f()
            for b in range(B):
                A = S2(b)
                A[0]()
                if b > 0:
                    epiB(b - 1)
                if b + 1 < B:
                    start_batch(b + 1)
                    Bl = S1(b + 1)
                    Bl[0](); Bl[1]()
                    A[1]()
                    Bl[2](); Bl[3]()
                    A[2]()
                    Bl[4](); Bl[5]()
                    A[3]()
                    Bl[6](); Bl[7]()
                    A[4]()
                    Bl[8](); Bl[9](); Bl[10](); Bl[11](); Bl[12](); Bl[13]()
                    A[5]()
                else:
                    for f in A[1:]:
                        f()
            epiB(B - 1)

    _t1 = _time.time()
    nc.compile()
    print(f"[kernel] tile-trace+schedule {_t1 - _t0:.1f}s, "
          f"bacc compile {_time.time() - _t1:.1f}s", file=sys.stderr, flush=True)
    return nc


def get_nc():
    if "nc" not in _CACHE:
        _CACHE["nc"] = _build_nc()
    return _CACHE["nc"]


def make_in_maps(x, W1, Wo1, W2, Wo2):
    import ml_dtypes
    bf = ml_dtypes.bfloat16
    x = np.ascontiguousarray(np.asarray(x, dtype=np.float32))
    xT = np.ascontiguousarray(x.transpose(0, 2, 1)).astype(bf)
    xb = x.astype(bf)
    w1t = np.ascontiguousarray(np.asarray(W1, np.float32).T).astype(bf)
    wo1t = np.ascontiguousarray(np.asarray(Wo1, np.float32).T).astype(bf)
    w2t = np.ascontiguousarray(np.asarray(W2, np.float32).T).astype(bf)
    # 1/L mean-pooling factor folded into the stage-2 output projection
    wo2t = (np.ascontiguousarray(np.asarray(Wo2, np.float32).T)
            * (1.0 / L)).astype(bf)
    ident = np.eye(P, dtype=np.float32).astype(bf)
    onesc = np.ones((P, 1), dtype=np.float32).astype(bf)
    onesr = np.ones((1, P), dtype=np.float32).astype(bf)
    return [
        {"x": xb[c * B:(c + 1) * B], "xT": xT[c * B:(c + 1) * B],
         "w1t": w1t, "wo1t": wo1t, "w2t": w2t, "wo2t": wo2t,
         "ident": ident, "onesc": onesc, "onesr": onesr}
        for c in range(NCORES)
    ]


def run(x, W1, Wo1, W2, Wo2, trace=False, **kw):
    from concourse.bass_utils import run_bass_kernel_spmd
    nc = get_nc()
    in_maps = make_in_maps(x, W1, Wo1, W2, Wo2)
    res = run_bass_kernel_spmd(nc, in_maps, core_ids=list(range(NCORES)),
                               trace=trace, **kw)
    out = np.concatenate([res.results[c]["out"] for c in range(NCORES)], axis=0)
    return out.reshape(N_GLOBAL, D, 1, 1), res


def kernel(**inputs):
    out, _ = run(inputs["x"], inputs["W1"], inputs["Wo1"],
                 inputs["W2"], inputs["Wo2"])
    return out
